# revision 29
# baseline (speedup 1.0000x reference)
"""Chamfer loss kernel for Trainium2 (Bass/Tile), 8-core data-parallel.

Per core (one batch element): full pairwise squared distances are formed
directly in PSUM by a single K=13 bf16 matmul pass per tile using a hi/lo
bf16 split of the fp32 coordinates:

  d(n, m) = -2*(xh.yh + xh.yl + xl.yh) + (y2h + y2l) + (x2h + x2l)

(the xl.yl term is below fp32 noise for these magnitudes). PSUM holds the
full distance values in fp32; DVE tensor_reduce takes the row-min over
the candidates, then relu + row-sum produce a [128, 2] per-core partial
that the host combines into the weighted batch mean.
"""

import sys

import numpy as np

for _p in ("/opt/trn_rl_repo",):
    if _p not in sys.path:
        sys.path.insert(0, _p)

import ml_dtypes
from contextlib import ExitStack

# The agent image's antenv package lacks the axon_hooks module that
# concourse.bass_utils imports for trace=True runs under axon.  Synthesize
# it (same ctypes NTFF hook the trn boot installs when the module exists).
def _ensure_axon_hooks():
    import types
    import ctypes
    import contextlib

    try:
        import antenv.axon_hooks  # noqa: F401
        return
    except ImportError:
        pass
    mod = types.ModuleType("antenv.axon_hooks")
    state = {"hook": None}
    mod.set_axon_ntff_profile_hook = lambda h: state.__setitem__("hook", h)
    mod.get_axon_ntff_profile_hook = lambda: state["hook"]
    sys.modules["antenv.axon_hooks"] = mod
    import antenv
    antenv.axon_hooks = mod

    so_path = "/opt/axon/libaxon_pjrt.so"
    try:
        lib = ctypes.CDLL(so_path)
    except OSError:
        return
    if not hasattr(lib, "axon_start_nrt_profile"):
        return
    lib.axon_start_nrt_profile.argtypes = [ctypes.POINTER(ctypes.c_int64),
                                           ctypes.c_size_t]
    lib.axon_start_nrt_profile.restype = ctypes.c_int64
    lib.axon_stop_nrt_profile.argtypes = [ctypes.c_char_p]
    lib.axon_stop_nrt_profile.restype = ctypes.c_int64

    @contextlib.contextmanager
    def _hook(output_dir, device_ids):
        import jax
        jax.devices()
        if device_ids:
            ids = (ctypes.c_int64 * len(device_ids))(*device_ids)
            rc = lib.axon_start_nrt_profile(ids, len(device_ids))
        else:
            rc = lib.axon_start_nrt_profile(None, 0)
        if rc != 0:
            raise RuntimeError(f"axon_start_nrt_profile rc={rc}")
        try:
            yield
        finally:
            n = lib.axon_stop_nrt_profile(str(output_dir).encode())
            print(f"profile: {n} file(s) written to {output_dir}",
                  file=sys.stderr)

    mod.set_axon_ntff_profile_hook(_hook)


_ensure_axon_hooks()

import concourse.bass as bass
import concourse.bacc as bacc
import concourse.tile as tile
from concourse import mybir
from concourse.bass_utils import run_bass_kernel_spmd

BF16 = ml_dtypes.bfloat16
B, N_PTS, M_PTS = 8, 4096, 4096
N_CORES = 8
FMAX = 3.0e38


def build_program(n_pts=N_PTS, m_pts=M_PTS, trace_sim=False, direct_every=0,
                  n_strips=4):
    """Build + compile the single-core Bass program (SPMD across 8 cores).

    direct_every: 0 -> every point-block is min-reduced by DVE straight off
    PSUM (1x mode).  k > 0 -> only every k-th block goes direct; the rest are
    converted fp32->bf16 by the scalar engine first so DVE runs its 2x
    tensor_tensor min path, splitting the reduction load across ACT + DVE.
    """
    assert n_pts % 2048 == 0 and m_pts % 2048 == 0

    f32 = mybir.dt.float32
    bf = mybir.dt.bfloat16
    MIN = mybir.AluOpType.min
    ADD = mybir.AluOpType.add

    nc = bacc.Bacc("TRN2", target_bir_lowering=False, debug=False,
                   enable_asserts=False)
    xth = nc.dram_tensor("xth", [3, n_pts], bf, kind="ExternalInput").ap()
    xtl = nc.dram_tensor("xtl", [3, n_pts], bf, kind="ExternalInput").ap()
    yth = nc.dram_tensor("yth", [3, m_pts], bf, kind="ExternalInput").ap()
    ytl = nc.dram_tensor("ytl", [3, m_pts], bf, kind="ExternalInput").ap()
    xn = nc.dram_tensor("xn", [128, 3 * (n_pts // 128)], f32,
                        kind="ExternalInput").ap()
    yn = nc.dram_tensor("yn", [128, 3 * (m_pts // 128)], f32,
                        kind="ExternalInput").ap()
    out = nc.dram_tensor("out", [128, 2], f32, kind="ExternalOutput").ap()

    with tile.TileContext(nc, trace_sim=trace_sim) as tc, ExitStack() as ctx:
        const = ctx.enter_context(tc.tile_pool(name="const", bufs=1))
        psum = ctx.enter_context(tc.tile_pool(name="psum", bufs=2, space="PSUM"))
        work = ctx.enter_context(tc.tile_pool(name="work", bufs=3))

        xh_s = const.tile([3, n_pts], bf, tag="xh_s")
        nc.sync.dma_start(xh_s[:], xth)
        xl_s = const.tile([3, n_pts], bf, tag="xl_s")
        nc.sync.dma_start(xl_s[:], xtl)
        yh_s = const.tile([3, m_pts], bf, tag="yh_s")
        nc.sync.dma_start(yh_s[:], yth)
        yl_s = const.tile([3, m_pts], bf, tag="yl_s")
        nc.sync.dma_start(yl_s[:], ytl)
        xn_s = const.tile([128, 3 * (n_pts // 128)], f32, tag="xn_s")
        nc.sync.dma_start(xn_s[:], xn)
        yn_s = const.tile([128, 3 * (m_pts // 128)], f32, tag="yn_s")
        nc.sync.dma_start(yn_s[:], yn)

        # Per point set: squared norms in natural layout, then the two matmul
        # operand forms.  Compute-engine SBUF APs must start at partition
        # 0/32/64/96 (BIR verifier), so all row placement into the 13-row
        # operand tiles goes through SBUF->SBUF DMA.
        ones = const.tile([2, max(n_pts, m_pts)], bf, tag="ones")
        nc.vector.memset(ones[:], 1.0)

        forms = []
        for nm, h_s, l_s, n_s, npts in (("x", xh_s, xl_s, xn_s, n_pts),
                                        ("y", yh_s, yl_s, yn_s, m_pts)):
            nblk = npts // 128
            sq = const.tile([128, 3 * nblk], f32, tag=f"sq_{nm}")
            nc.vector.tensor_mul(sq[:], n_s[:], n_s[:])
            sq3 = sq[:].rearrange("p (j d) -> p j d", d=3)
            s2 = const.tile([128, nblk], f32, tag=f"s2_{nm}")
            nc.vector.tensor_add(s2[:], sq3[:, :, 0], sq3[:, :, 1])
            nc.vector.tensor_add(s2[:], s2[:], sq3[:, :, 2])
            s2h = const.tile([128, nblk], bf, tag=f"s2h_{nm}")
            nc.vector.tensor_copy(s2h[:], s2[:])
            s2l = const.tile([128, nblk], bf, tag=f"s2l_{nm}")
            nc.vector.tensor_sub(s2l[:], s2[:], s2h[:])

            n2h = const.tile([3, npts], bf, tag=f"n2h_{nm}")
            nc.vector.tensor_scalar_mul(n2h[:], h_s[:], -2.0)
            n2l = const.tile([3, npts], bf, tag=f"n2l_{nm}")
            nc.vector.tensor_scalar_mul(n2l[:], l_s[:], -2.0)

            # stationary form: [-2h x3, -2h x3, -2l x3, 1, 1, s2h, s2l],
            # duplicated at partitions 32-44 so consecutive blocks hit
            # different PE row-groups (LDWEIGHTS then overlaps MATMUL).
            st = const.tile([32 * 3 + 13, npts], bf, tag=f"st_{nm}")
            nc.sync.dma_start(st[0:3, :], n2h[:])
            nc.sync.dma_start(st[3:6, :], n2h[:])
            nc.sync.dma_start(st[6:9, :], n2l[:])
            nc.sync.dma_start(st[9:11, :], ones[:, :npts])
            nc.sync.dma_start(st[11:12, :], s2h[:])
            nc.sync.dma_start(st[12:13, :], s2l[:])
            for s in range(1, n_strips):
                nc.sync.dma_start(st[32 * s:32 * s + 13, :], st[0:13, :])

            # moving form: [h x3, l x3, h x3, s2h, s2l, 1, 1]
            mv = const.tile([32 * 3 + 13, npts], bf, tag=f"mv_{nm}")
            nc.sync.dma_start(mv[0:3, :], h_s[:])
            nc.sync.dma_start(mv[3:6, :], l_s[:])
            nc.sync.dma_start(mv[6:9, :], h_s[:])
            nc.sync.dma_start(mv[9:10, :], s2h[:])
            nc.sync.dma_start(mv[10:11, :], s2l[:])
            nc.sync.dma_start(mv[11:13, :], ones[:, :npts])
            for s in range(1, n_strips):
                nc.sync.dma_start(mv[32 * s:32 * s + 13, :], mv[0:13, :])
            forms.append((st, mv))

        res = const.tile([128, 2], f32, tag="res")
        npts_of = {"x": n_pts, "y": m_pts}
        for d, (qi, ti) in enumerate(((0, 1), (1, 0))):
            st = forms[qi][0]
            mv = forms[ti][1]
            nj = npts_of["x" if d == 0 else "y"] // 128   # stationary blocks
            mh = npts_of["y" if d == 0 else "x"] // 2048  # moving psum tiles
            mpts = npts_of["y" if d == 0 else "x"]
            nwaves = mpts // 2048     # 4-bank psum tiles per block
            rm2 = [const.tile([128, nj], f32, tag=f"rm2_{d}_{u}",
                              name=f"rm2_{d}_{u}")
                   for u in range(max(nwaves, 2))]
            for u in range(1, len(rm2)):
                nc.gpsimd.memset(rm2[u][:], FMAX)
            for j in range(nj):
                direct = direct_every == 0 or (j % direct_every == 0)
                cols = slice(j * 128, (j + 1) * 128)
                pts = []
                for w in range(nwaves):
                    pt = psum.tile([128, 2048], f32, tag="pt")
                    for q in range(4):
                        s = (4 * w + q) % n_strips   # PE row-strip
                        mo = w * 2048 + q * 512
                        nc.tensor.matmul(pt[:, q * 512:(q + 1) * 512],
                                         st[32 * s:32 * s + 13, cols],
                                         mv[32 * s:32 * s + 13, mo:mo + 512],
                                         start=True, stop=True,
                                         tile_position=(32 * s, 0))
                    if direct:
                        nc.vector.tensor_reduce(
                            rm2[w][:, j:j + 1], pt[:],
                            axis=mybir.AxisListType.X, op=MIN)
                    else:
                        pts.append(pt)
                if not direct:
                    # ACT converts psum fp32 -> sbuf bf16; DVE min-combines
                    # at 2x, trees down to 256 wide, then one 1x reduce.
                    sbs = []
                    for u, pt in enumerate(pts):
                        sb = work.tile([128, 2048], bf, tag=f"sb{u}")
                        nc.scalar.copy(sb[:], pt[:])
                        sbs.append(sb)
                    tm = sbs[0]
                    for u in range(1, nwaves):
                        nc.vector.tensor_tensor(tm[:], tm[:], sbs[u][:], op=MIN)
                    nc.vector.tensor_tensor(tm[:, 0:1024], tm[:, 0:1024],
                                            tm[:, 1024:2048], op=MIN)
                    nc.vector.tensor_tensor(tm[:, 0:512], tm[:, 0:512],
                                            tm[:, 512:1024], op=MIN)
                    nc.vector.tensor_tensor(tm[:, 0:256], tm[:, 0:256],
                                            tm[:, 256:512], op=MIN)
                    nc.vector.tensor_reduce(rm2[0][:, j:j + 1], tm[:, 0:256],
                                            axis=mybir.AxisListType.X, op=MIN)
            rm = const.tile([128, nj], f32, tag=f"rm{d}")
            nc.vector.tensor_tensor(rm[:], rm2[0][:], rm2[1][:], op=MIN)
            for u in range(2, len(rm2)):
                nc.vector.tensor_tensor(rm[:], rm[:], rm2[u][:], op=MIN)
            nc.vector.tensor_scalar_max(rm[:], rm[:], 0.0)
            nc.vector.tensor_reduce(res[:, d:d + 1], rm[:],
                                    axis=mybir.AxisListType.X, op=ADD)
        nc.sync.dma_start(out, res[:])

    nc.compile()
    return nc


def _split_t(a):
    """(P, 3) fp32 -> ([3, P], [3, P]) bf16 transposed hi/lo rows."""
    hi = a.astype(BF16)
    lo = (a - hi.astype(np.float32)).astype(BF16)
    return np.ascontiguousarray(hi.T), np.ascontiguousarray(lo.T)


def _in_map(pts_x, pts_y):
    nb = pts_x.shape[0] // 128
    mb = pts_y.shape[0] // 128
    xth, xtl = _split_t(pts_x)
    yth, ytl = _split_t(pts_y)
    return {
        "xth": xth, "xtl": xtl, "yth": yth, "ytl": ytl,
        "xn": np.ascontiguousarray(pts_x.reshape(128, 3 * nb)),
        "yn": np.ascontiguousarray(pts_y.reshape(128, 3 * mb)),
    }


_PROGRAM = None
TRACE = False          # set True (e.g. from test.py) to capture an NTFF profile
LAST_RESULT = None     # BassKernelResults of the most recent run
DIRECT_EVERY = 5       # reduction lane split; see build_program()
N_STRIPS = 4           # concurrent PE row-strips


def kernel(x, y, weight):
    global _PROGRAM, LAST_RESULT
    x = np.asarray(x, dtype=np.float32)
    y = np.asarray(y, dtype=np.float32)
    w = np.asarray(weight, dtype=np.float32)
    if _PROGRAM is None:
        _PROGRAM = build_program(direct_every=DIRECT_EVERY, n_strips=N_STRIPS)
    in_maps = [_in_map(x[b], y[b]) for b in range(B)]
    res = run_bass_kernel_spmd(_PROGRAM, in_maps, list(range(N_CORES)),
                               trace=TRACE)
    LAST_RESULT = res
    losses = np.zeros(B, dtype=np.float64)
    for b in range(B):
        o = res.results[b]["out"].astype(np.float64)
        losses[b] = o[:, 0].sum() / N_PTS + o[:, 1].sum() / M_PTS
    total = (losses * w.astype(np.float64)).mean()
    return np.float32(total)


# revision 30
# speedup vs baseline: 1.0164x; 1.0164x over previous
"""Chamfer loss kernel for Trainium2 (Bass/Tile), 8-core data-parallel.

Per core (one batch element): full pairwise squared distances are formed
directly in PSUM by a single K=13 bf16 matmul pass per tile using a hi/lo
bf16 split of the fp32 coordinates:

  d(n, m) = -2*(xh.yh + xh.yl + xl.yh) + (y2h + y2l) + (x2h + x2l)

(the xl.yl term is below fp32 noise for these magnitudes). PSUM holds the
full distance values in fp32; DVE tensor_reduce takes the row-min over
the candidates, then relu + row-sum produce a [128, 2] per-core partial
that the host combines into the weighted batch mean.
"""

import sys

import numpy as np

for _p in ("/opt/trn_rl_repo",):
    if _p not in sys.path:
        sys.path.insert(0, _p)

import ml_dtypes
from contextlib import ExitStack

# The agent image's antenv package lacks the axon_hooks module that
# concourse.bass_utils imports for trace=True runs under axon.  Synthesize
# it (same ctypes NTFF hook the trn boot installs when the module exists).
def _ensure_axon_hooks():
    import types
    import ctypes
    import contextlib

    try:
        import antenv.axon_hooks  # noqa: F401
        return
    except ImportError:
        pass
    mod = types.ModuleType("antenv.axon_hooks")
    state = {"hook": None}
    mod.set_axon_ntff_profile_hook = lambda h: state.__setitem__("hook", h)
    mod.get_axon_ntff_profile_hook = lambda: state["hook"]
    sys.modules["antenv.axon_hooks"] = mod
    import antenv
    antenv.axon_hooks = mod

    so_path = "/opt/axon/libaxon_pjrt.so"
    try:
        lib = ctypes.CDLL(so_path)
    except OSError:
        return
    if not hasattr(lib, "axon_start_nrt_profile"):
        return
    lib.axon_start_nrt_profile.argtypes = [ctypes.POINTER(ctypes.c_int64),
                                           ctypes.c_size_t]
    lib.axon_start_nrt_profile.restype = ctypes.c_int64
    lib.axon_stop_nrt_profile.argtypes = [ctypes.c_char_p]
    lib.axon_stop_nrt_profile.restype = ctypes.c_int64

    @contextlib.contextmanager
    def _hook(output_dir, device_ids):
        import jax
        jax.devices()
        if device_ids:
            ids = (ctypes.c_int64 * len(device_ids))(*device_ids)
            rc = lib.axon_start_nrt_profile(ids, len(device_ids))
        else:
            rc = lib.axon_start_nrt_profile(None, 0)
        if rc != 0:
            raise RuntimeError(f"axon_start_nrt_profile rc={rc}")
        try:
            yield
        finally:
            n = lib.axon_stop_nrt_profile(str(output_dir).encode())
            print(f"profile: {n} file(s) written to {output_dir}",
                  file=sys.stderr)

    mod.set_axon_ntff_profile_hook(_hook)


_ensure_axon_hooks()

import concourse.bass as bass
import concourse.bacc as bacc
import concourse.tile as tile
from concourse import mybir
from concourse.bass_utils import run_bass_kernel_spmd

BF16 = ml_dtypes.bfloat16
B, N_PTS, M_PTS = 8, 4096, 4096
N_CORES = 8
FMAX = 3.0e38


def build_program(n_pts=N_PTS, m_pts=M_PTS, trace_sim=False, direct_every=0,
                  n_strips=4):
    """Build + compile the single-core Bass program (SPMD across 8 cores).

    direct_every: 0 -> every point-block is min-reduced by DVE straight off
    PSUM (1x mode).  k > 0 -> only every k-th block goes direct; the rest are
    converted fp32->bf16 by the scalar engine first so DVE runs its 2x
    tensor_tensor min path, splitting the reduction load across ACT + DVE.
    """
    assert n_pts % 2048 == 0 and m_pts % 2048 == 0

    f32 = mybir.dt.float32
    bf = mybir.dt.bfloat16
    MIN = mybir.AluOpType.min
    ADD = mybir.AluOpType.add

    nc = bacc.Bacc("TRN2", target_bir_lowering=False, debug=False,
                   enable_asserts=False)
    xth = nc.dram_tensor("xth", [3, n_pts], bf, kind="ExternalInput").ap()
    xtl = nc.dram_tensor("xtl", [3, n_pts], bf, kind="ExternalInput").ap()
    yth = nc.dram_tensor("yth", [3, m_pts], bf, kind="ExternalInput").ap()
    ytl = nc.dram_tensor("ytl", [3, m_pts], bf, kind="ExternalInput").ap()
    xn = nc.dram_tensor("xn", [128, 3 * (n_pts // 128)], f32,
                        kind="ExternalInput").ap()
    yn = nc.dram_tensor("yn", [128, 3 * (m_pts // 128)], f32,
                        kind="ExternalInput").ap()
    out = nc.dram_tensor("out", [128, 2], f32, kind="ExternalOutput").ap()

    with tile.TileContext(nc, trace_sim=trace_sim) as tc, ExitStack() as ctx:
        const = ctx.enter_context(tc.tile_pool(name="const", bufs=1))
        psum = ctx.enter_context(tc.tile_pool(name="psum", bufs=4, space="PSUM"))
        work = ctx.enter_context(tc.tile_pool(name="work", bufs=3))

        xh_s = const.tile([3, n_pts], bf, tag="xh_s")
        nc.sync.dma_start(xh_s[:], xth)
        xl_s = const.tile([3, n_pts], bf, tag="xl_s")
        nc.sync.dma_start(xl_s[:], xtl)
        yh_s = const.tile([3, m_pts], bf, tag="yh_s")
        nc.sync.dma_start(yh_s[:], yth)
        yl_s = const.tile([3, m_pts], bf, tag="yl_s")
        nc.sync.dma_start(yl_s[:], ytl)
        xn_s = const.tile([128, 3 * (n_pts // 128)], f32, tag="xn_s")
        nc.sync.dma_start(xn_s[:], xn)
        yn_s = const.tile([128, 3 * (m_pts // 128)], f32, tag="yn_s")
        nc.sync.dma_start(yn_s[:], yn)

        # Per point set: squared norms in natural layout, then the two matmul
        # operand forms.  Compute-engine SBUF APs must start at partition
        # 0/32/64/96 (BIR verifier), so all row placement into the 13-row
        # operand tiles goes through SBUF->SBUF DMA.
        ones = const.tile([2, max(n_pts, m_pts)], bf, tag="ones")
        nc.vector.memset(ones[:], 1.0)

        forms = []
        for nm, h_s, l_s, n_s, npts in (("x", xh_s, xl_s, xn_s, n_pts),
                                        ("y", yh_s, yl_s, yn_s, m_pts)):
            nblk = npts // 128
            sq = const.tile([128, 3 * nblk], f32, tag=f"sq_{nm}")
            nc.vector.tensor_mul(sq[:], n_s[:], n_s[:])
            sq3 = sq[:].rearrange("p (j d) -> p j d", d=3)
            s2 = const.tile([128, nblk], f32, tag=f"s2_{nm}")
            nc.vector.tensor_add(s2[:], sq3[:, :, 0], sq3[:, :, 1])
            nc.vector.tensor_add(s2[:], s2[:], sq3[:, :, 2])
            s2h = const.tile([128, nblk], bf, tag=f"s2h_{nm}")
            nc.vector.tensor_copy(s2h[:], s2[:])
            s2l = const.tile([128, nblk], bf, tag=f"s2l_{nm}")
            nc.vector.tensor_sub(s2l[:], s2[:], s2h[:])

            n2h = const.tile([3, npts], bf, tag=f"n2h_{nm}")
            nc.vector.tensor_scalar_mul(n2h[:], h_s[:], -2.0)
            n2l = const.tile([3, npts], bf, tag=f"n2l_{nm}")
            nc.vector.tensor_scalar_mul(n2l[:], l_s[:], -2.0)

            # stationary form: [-2h x3, -2h x3, -2l x3, 1, 1, s2h, s2l],
            # duplicated at partitions 32-44 so consecutive blocks hit
            # different PE row-groups (LDWEIGHTS then overlaps MATMUL).
            st = const.tile([32 * 3 + 13, npts], bf, tag=f"st_{nm}")
            nc.sync.dma_start(st[0:3, :], n2h[:])
            nc.sync.dma_start(st[3:6, :], n2h[:])
            nc.sync.dma_start(st[6:9, :], n2l[:])
            nc.sync.dma_start(st[9:11, :], ones[:, :npts])
            nc.sync.dma_start(st[11:12, :], s2h[:])
            nc.sync.dma_start(st[12:13, :], s2l[:])
            for s in range(1, n_strips):
                nc.sync.dma_start(st[32 * s:32 * s + 13, :], st[0:13, :])

            # moving form: [h x3, l x3, h x3, s2h, s2l, 1, 1]
            mv = const.tile([32 * 3 + 13, npts], bf, tag=f"mv_{nm}")
            nc.sync.dma_start(mv[0:3, :], h_s[:])
            nc.sync.dma_start(mv[3:6, :], l_s[:])
            nc.sync.dma_start(mv[6:9, :], h_s[:])
            nc.sync.dma_start(mv[9:10, :], s2h[:])
            nc.sync.dma_start(mv[10:11, :], s2l[:])
            nc.sync.dma_start(mv[11:13, :], ones[:, :npts])
            for s in range(1, n_strips):
                nc.sync.dma_start(mv[32 * s:32 * s + 13, :], mv[0:13, :])
            forms.append((st, mv))

        res = const.tile([128, 2], f32, tag="res")
        npts_of = {"x": n_pts, "y": m_pts}
        for d, (qi, ti) in enumerate(((0, 1), (1, 0))):
            st = forms[qi][0]
            mv = forms[ti][1]
            nj = npts_of["x" if d == 0 else "y"] // 128   # stationary blocks
            mh = npts_of["y" if d == 0 else "x"] // 2048  # moving psum tiles
            mpts = npts_of["y" if d == 0 else "x"]
            units = mpts // 1024      # 2-bank psum tiles per block
            rm2 = [const.tile([128, nj], f32, tag=f"rm2_{d}_{u}",
                              name=f"rm2_{d}_{u}")
                   for u in range(min(units, 4))]
            for u in range(1, len(rm2)):
                nc.gpsimd.memset(rm2[u][:], FMAX)
            for j in range(nj):
                direct = direct_every == 0 or (j % direct_every == 0)
                cols = slice(j * 128, (j + 1) * 128)
                pts = []
                for u in range(units):
                    pt = psum.tile([128, 1024], f32, tag="pt")
                    for q in range(2):
                        s = (2 * u + q) % n_strips   # PE row-strip
                        mo = u * 1024 + q * 512
                        nc.tensor.matmul(pt[:, q * 512:(q + 1) * 512],
                                         st[32 * s:32 * s + 13, cols],
                                         mv[32 * s:32 * s + 13, mo:mo + 512],
                                         start=True, stop=True,
                                         tile_position=(32 * s, 0))
                    if direct:
                        nc.vector.tensor_reduce(
                            rm2[u % 4][:, j:j + 1], pt[:],
                            axis=mybir.AxisListType.X, op=MIN)
                    else:
                        pts.append(pt)
                if not direct:
                    # ACT converts psum fp32 -> sbuf bf16; DVE min-combines
                    # at 2x, trees down to 256 wide, then one 1x reduce.
                    sbs = []
                    for u, pt in enumerate(pts):
                        sb = work.tile([128, 1024], bf, tag=f"sb{u}")
                        nc.scalar.copy(sb[:], pt[:])
                        sbs.append(sb)
                    tm = sbs[0]
                    for u in range(1, units):
                        nc.vector.tensor_tensor(tm[:], tm[:], sbs[u][:], op=MIN)
                    nc.vector.tensor_tensor(tm[:, 0:512], tm[:, 0:512],
                                            tm[:, 512:1024], op=MIN)
                    nc.vector.tensor_tensor(tm[:, 0:256], tm[:, 0:256],
                                            tm[:, 256:512], op=MIN)
                    nc.vector.tensor_reduce(rm2[0][:, j:j + 1], tm[:, 0:256],
                                            axis=mybir.AxisListType.X, op=MIN)
            rm = const.tile([128, nj], f32, tag=f"rm{d}")
            nc.vector.tensor_tensor(rm[:], rm2[0][:], rm2[1][:], op=MIN)
            for u in range(2, len(rm2)):
                nc.vector.tensor_tensor(rm[:], rm[:], rm2[u][:], op=MIN)
            nc.vector.tensor_scalar_max(rm[:], rm[:], 0.0)
            nc.vector.tensor_reduce(res[:, d:d + 1], rm[:],
                                    axis=mybir.AxisListType.X, op=ADD)
        nc.sync.dma_start(out, res[:])

    nc.compile()
    return nc


def _split_t(a):
    """(P, 3) fp32 -> ([3, P], [3, P]) bf16 transposed hi/lo rows."""
    hi = a.astype(BF16)
    lo = (a - hi.astype(np.float32)).astype(BF16)
    return np.ascontiguousarray(hi.T), np.ascontiguousarray(lo.T)


def _in_map(pts_x, pts_y):
    nb = pts_x.shape[0] // 128
    mb = pts_y.shape[0] // 128
    xth, xtl = _split_t(pts_x)
    yth, ytl = _split_t(pts_y)
    return {
        "xth": xth, "xtl": xtl, "yth": yth, "ytl": ytl,
        "xn": np.ascontiguousarray(pts_x.reshape(128, 3 * nb)),
        "yn": np.ascontiguousarray(pts_y.reshape(128, 3 * mb)),
    }


_PROGRAM = None
TRACE = False          # set True (e.g. from test.py) to capture an NTFF profile
LAST_RESULT = None     # BassKernelResults of the most recent run
DIRECT_EVERY = 8       # reduction lane split; see build_program()
N_STRIPS = 4           # concurrent PE row-strips


def kernel(x, y, weight):
    global _PROGRAM, LAST_RESULT
    x = np.asarray(x, dtype=np.float32)
    y = np.asarray(y, dtype=np.float32)
    w = np.asarray(weight, dtype=np.float32)
    if _PROGRAM is None:
        _PROGRAM = build_program(direct_every=DIRECT_EVERY, n_strips=N_STRIPS)
    in_maps = [_in_map(x[b], y[b]) for b in range(B)]
    res = run_bass_kernel_spmd(_PROGRAM, in_maps, list(range(N_CORES)),
                               trace=TRACE)
    LAST_RESULT = res
    losses = np.zeros(B, dtype=np.float64)
    for b in range(B):
        o = res.results[b]["out"].astype(np.float64)
        losses[b] = o[:, 0].sum() / N_PTS + o[:, 1].sum() / M_PTS
    total = (losses * w.astype(np.float64)).mean()
    return np.float32(total)
